# revision 3
# baseline (speedup 1.0000x reference)
"""Multi-head attention (B=2, S=2048, D=1024, H=16) on 8 Trainium2 NeuronCores.

Sharding: core c -> batch b = c // 4, head-group g = c % 4 (4 heads = 256 proj
dims per core). Each core computes its 4 heads' attention plus the matching
slice of the output projection; the host sums the 4 partial outputs per batch
and adds bo.

Device layouts (all matmul operands float32r = fp32 bits, 1 cyc/row on PE):
  qT/kT [o, s]   : proj from host-transposed Q/K (contraction dim on partitions)
  v     [s, o]   : natural layout + ones column per head (softmax denominator
                   rides along row 64 of the PV matmul output)
  scoresT [k, q] : lhsT = kT head slice -> softmax reduction via the ones col
  outT  [d, q]   : unnormalized; normalized by 1/denom broadcast via a K=1 matmul
  out_pT [o, q]  : local slice of x @ Wo.T, host transposes + sums + bias
"""

import numpy as np

import concourse.bass as bass
import concourse.mybir as mybir
import concourse.tile as tile
from concourse import bacc
from concourse.bass_utils import run_bass_kernel_spmd

B, S, D, H = 2, 2048, 1024, 16
OL = 256          # local projection dims (4 heads x 64)
NI = D // 128     # contraction chunks for projections
NK = S // 128     # key chunks
NQ = S // 512     # query blocks
DT = None         # set in _build (mybir import scope)
F32 = None

_CACHE = {}


def _build():
    DT = mybir.dt.float32r
    F32 = mybir.dt.float32
    AF = mybir.ActivationFunctionType

    nc = bacc.Bacc("TRN2", target_bir_lowering=False, debug=False, num_devices=8)

    qt_d = nc.dram_tensor("qt", [D, S], DT, kind="ExternalInput").ap() \
        .rearrange("(c p) s -> c p s", p=128)
    kt_d = nc.dram_tensor("kt", [D, S], DT, kind="ExternalInput").ap() \
        .rearrange("(c p) s -> c p s", p=128)
    vt_d = nc.dram_tensor("vt", [D, S], DT, kind="ExternalInput").ap() \
        .rearrange("(c p) s -> c p s", p=128)
    wq_d = nc.dram_tensor("wqt", [D, OL], DT, kind="ExternalInput").ap() \
        .rearrange("(c p) o -> c p o", p=128)
    wk_d = nc.dram_tensor("wkt", [D, OL], DT, kind="ExternalInput").ap() \
        .rearrange("(c p) o -> c p o", p=128)
    wv_d = nc.dram_tensor("wvt", [D, OL], DT, kind="ExternalInput").ap() \
        .rearrange("(c p) o -> c p o", p=128)
    bq_d = nc.dram_tensor("bq2", [2, 128, 1], F32, kind="ExternalInput").ap()
    bk_d = nc.dram_tensor("bk2", [2, 128, 1], F32, kind="ExternalInput").ap()
    bv_d = nc.dram_tensor("bv1", [1, OL], DT, kind="ExternalInput").ap()
    wo_d = nc.dram_tensor("wot", [OL, D], DT, kind="ExternalInput").ap() \
        .rearrange("(c p) o -> c p o", p=128)
    out_d = nc.dram_tensor("out_t", [D, S], F32, kind="ExternalOutput").ap() \
        .rearrange("(c p) s -> c p s", p=128)

    with tile.TileContext(nc) as tc:
        with (
            tc.tile_pool(name="per", bufs=1) as per,
            tc.tile_pool(name="wp", bufs=1) as wp,
            tc.tile_pool(name="ip", bufs=1) as ip,
            tc.tile_pool(name="pr", bufs=4) as pr,
            tc.tile_pool(name="sm", bufs=4) as sm,
            tc.tile_pool(name="ot", bufs=2) as ot,
            tc.tile_pool(name="osg", bufs=3) as osg,
            tc.tile_pool(name="pj", bufs=1, space="PSUM") as pj,
            tc.tile_pool(name="p1", bufs=2, space="PSUM") as p1,
            tc.tile_pool(name="p2", bufs=2, space="PSUM") as p2,
            tc.tile_pool(name="pb", bufs=1, space="PSUM") as pb,
            tc.tile_pool(name="po", bufs=2, space="PSUM") as po,
        ):
            # --- persistent tiles
            qt_sb = [per.tile([128, S], DT, tag=f"qt{m}", name=f"qt{m}") for m in range(2)]
            kt_sb = [per.tile([128, S], DT, tag=f"kt{m}", name=f"kt{m}") for m in range(2)]
            v_sb = [per.tile([128, 4, 65], DT, tag=f"v{sc}", name=f"v{sc}") for sc in range(NK)]
            wo_sb = [per.tile([128, D], DT, tag=f"wo{c}", name=f"wo{c}") for c in range(2)]
            bq_sb = [per.tile([128, 1], F32, tag=f"bq{m}", name=f"bq{m}") for m in range(2)]
            bk_sb = [per.tile([128, 1], F32, tag=f"bk{m}", name=f"bk{m}") for m in range(2)]
            bv_sb = per.tile([1, OL], DT, tag="bv", name="bv")
            ones_r = per.tile([1, 128], DT, tag="ones_r", name="ones_r")
            ones_c = per.tile([1, 64], DT, tag="ones_c", name="ones_c")
            ones_f = per.tile([1, 128], F32, tag="ones_f", name="ones_f")
            vones_f = per.tile([128, 1], F32, tag="vones_f", name="vones_f")
            nc.vector.memset(vones_f[:], 1.0)
            nc.vector.memset(ones_f[:], 1.0)
            nc.vector.tensor_copy(ones_r[:], ones_f[:])
            nc.vector.tensor_copy(ones_c[:], ones_f[:, 0:64])

            for m in range(2):
                nc.sync.dma_start(bq_sb[m][:], bq_d[m])
                nc.sync.dma_start(bk_sb[m][:], bk_d[m])
            nc.sync.dma_start(bv_sb[:], bv_d)
            for c in range(2):
                nc.sync.dma_start(wo_sb[c][:], wo_d[c])

            def load_wa(w_dr, a_dr):
                ws, as_ = [], []
                for i in range(NI):
                    w = wp.tile([128, OL], DT, tag=f"w{i}", name=f"w{i}")
                    nc.sync.dma_start(w[:], w_dr[i])
                    ws.append(w)
                    a = ip.tile([128, S], DT, tag=f"a{i}", name=f"a{i}")
                    nc.sync.dma_start(a[:], a_dr[i])
                    as_.append(a)
                return ws, as_

            def proj_qk(w_dr, a_dr, bias_sb, dst_sb):
                # dst[o, s] = sum_i W[o, i] X[s, i]  (+ bias via ACT copy)
                ws, as_ = load_wa(w_dr, a_dr)
                for m in range(2):
                    for s in range(4):
                        ps = pj.tile([128, 512], F32, tag="ps", name="ps")
                        for i in range(NI):
                            nc.tensor.matmul(
                                ps[:],
                                ws[i][:, m * 128:(m + 1) * 128],
                                as_[i][:, s * 512:(s + 1) * 512],
                                start=(i == 0),
                                stop=(i == NI - 1),
                            )
                        nc.scalar.activation(
                            dst_sb[m][:, s * 512:(s + 1) * 512], ps[:],
                            AF.Identity, bias=bias_sb[m][:],
                        )

            def proj_v(w_dr, a_dr):
                # v[s, o] = sum_i X[s, i] W[o, i] + bv (bias via K=1 matmul)
                ws, as_ = load_wa(w_dr, a_dr)
                for sc in range(NK):
                    ps = pj.tile([128, OL], F32, tag="ps", name="ps")
                    for i in range(NI):
                        nc.tensor.matmul(
                            ps[:],
                            as_[i][:, sc * 128:(sc + 1) * 128],
                            ws[i][:],
                            start=(i == 0),
                            stop=False,
                        )
                    nc.tensor.matmul(
                        ps[:], ones_r[:], bv_sb[:], start=False, stop=True
                    )
                    for h in range(4):
                        nc.vector.tensor_copy(
                            v_sb[sc][:, h, 0:64], ps[:, h * 64:(h + 1) * 64]
                        )
                    for h in range(4):
                        nc.vector.tensor_copy(
                            v_sb[sc][:, h, 64:65], vones_f[:]
                        )

            proj_qk(wk_d, kt_d, bk_sb, kt_sb)
            proj_v(wv_d, vt_d)
            proj_qk(wq_d, qt_d, bq_sb, qt_sb)

            # --- attention + output projection, per query block
            for qb in range(NQ):
                qsl = slice(qb * 512, (qb + 1) * 512)
                ots = [ot.tile([128, 512], DT, tag=f"c{c}", name=f"otc{c}") for c in range(2)]
                for pair in range(2):
                    acc = [p2.tile([65, 512], F32, tag="acc", name="acc") for _ in range(2)]
                    pending = None  # software-pipelined PV matmuls
                    for kc in range(NK):
                        ksl = slice(kc * 128, (kc + 1) * 128)
                        prs = []
                        for hh in range(2):
                            psl = slice(hh * 64, (hh + 1) * 64)
                            ps1 = p1.tile([128, 512], F32, tag="s", name="s")
                            nc.tensor.matmul(
                                ps1[:], kt_sb[pair][psl, ksl],
                                qt_sb[pair][psl, qsl],
                                start=True, stop=True,
                            )
                            prob = pr.tile([128, 512], DT, tag="p", name="p")
                            nc.scalar.activation(
                                prob[:], ps1[:], AF.Exp, scale=0.125
                            )
                            prs.append(prob)
                        if pending is not None:
                            pkc, pprs = pending
                            for hh in range(2):
                                nc.tensor.matmul(
                                    acc[hh][:], v_sb[pkc][:, pair * 2 + hh, :],
                                    pprs[hh][:],
                                    start=(pkc == 0), stop=(pkc == NK - 1),
                                )
                        pending = (kc, prs)
                    pkc, pprs = pending
                    for hh in range(2):
                        nc.tensor.matmul(
                            acc[hh][:], v_sb[pkc][:, pair * 2 + hh, :],
                            pprs[hh][:],
                            start=(pkc == 0), stop=(pkc == NK - 1),
                        )
                    # normalize: outT rows = acc[0:64] * (1/acc[64]) bcast
                    for hh in range(2):
                        rec = sm.tile([1, 512], DT, tag="rec", name="rec")
                        with nc.allow_low_precision(reason="f32r matmul feed"):
                            nc.vector.reciprocal(rec[:], acc[hh][64:65, :])
                        rb_ps = pb.tile([64, 512], F32, tag="rb", name="rb")
                        nc.tensor.matmul(
                            rb_ps[:], ones_c[:], rec[:], start=True, stop=True
                        )
                        rb = sm.tile([64, 512], F32, tag="rb_sb", name="rb_sb")
                        nc.vector.tensor_copy(rb[:], rb_ps[:])
                        nc.vector.tensor_mul(
                            ots[pair][hh * 64:(hh + 1) * 64, :],
                            acc[hh][0:64, :], rb[:],
                        )
                # output projection for this query block
                for oc in range(8):
                    osl = slice(oc * 128, (oc + 1) * 128)
                    pso = po.tile([128, 512], F32, tag="o", name="o")
                    for c in range(2):
                        nc.tensor.matmul(
                            pso[:], wo_sb[c][:, osl], ots[c][:],
                            start=(c == 0), stop=(c == 1),
                        )
                    st = osg.tile([128, 512], F32, tag="st", name="st")
                    nc.vector.tensor_copy(st[:], pso[:])
                    nc.sync.dma_start(out_d[oc][:, qsl], st[:])

    nc.compile()
    return nc


def _get_nc():
    if "nc" not in _CACHE:
        _CACHE["nc"] = _build()
    return _CACHE["nc"]


def kernel(Q, K, V, Wq, bq, Wk, bk, Wv, bv, Wo, bo):
    nc = _get_nc()
    f = np.float32
    in_maps = []
    for core in range(8):
        b, g = divmod(core, 4)
        sl = slice(g * OL, (g + 1) * OL)
        in_maps.append({
            "qt": np.ascontiguousarray(Q[b].T, dtype=f),
            "kt": np.ascontiguousarray(K[b].T, dtype=f),
            "vt": np.ascontiguousarray(V[b].T, dtype=f),
            "wqt": np.ascontiguousarray(Wq[sl].T, dtype=f),
            "wkt": np.ascontiguousarray(Wk[sl].T, dtype=f),
            "wvt": np.ascontiguousarray(Wv[sl].T, dtype=f),
            "bq2": np.ascontiguousarray(bq[sl].reshape(2, 128, 1), dtype=f),
            "bk2": np.ascontiguousarray(bk[sl].reshape(2, 128, 1), dtype=f),
            "bv1": np.ascontiguousarray(bv[sl].reshape(1, OL), dtype=f),
            "wot": np.ascontiguousarray(Wo[:, sl].T, dtype=f),
        })
    res = run_bass_kernel_spmd(nc, in_maps, core_ids=list(range(8)))
    out = np.empty((B, S, D), np.float32)
    for b in range(B):
        acc = res.results[b * 4 + 0]["out_t"].astype(np.float64)
        for g in range(1, 4):
            acc += res.results[b * 4 + g]["out_t"]
        out[b] = (acc.T + bo).astype(np.float32)
    return out


# revision 4
# speedup vs baseline: 1.4306x; 1.4306x over previous
"""Multi-head attention (B=2, S=2048, D=1024, H=16) on 8 Trainium2 NeuronCores.

Sharding: core c -> batch b = c // 4, head-group g = c % 4 (4 heads = 256 proj
dims per core). Each core computes its 4 heads' attention plus the matching
slice of the output projection; the host sums the 4 partial outputs per batch
and adds bo.

Device layouts (matmul operands float32r = fp32 bits at bf16 PE rate):
  qT/kT [o, s]   : proj from host-transposed Q/K (contraction on partitions)
  v     [s, o]   : natural layout + ones column per head (softmax denominator
                   rides along row 64 of the PV matmul output)
  scoresT [k, q] : head pairs row-packed on the PE (base_partition 0/64);
                   both halves of a [128,1024] PSUM tile -> one wide Exp
  outT  [d, q]   : unnormalized; moved off PSUM fast, normalized with
                   reciprocal_approx_fast + GpSimd partition_broadcast
  out_pT [o, q]  : local slice of x @ Wo.T; host transposes + sums + bias
"""

import numpy as np

import concourse.bass as bass
import concourse.mybir as mybir
import concourse.tile as tile
from concourse import bacc
from concourse.bass_utils import run_bass_kernel_spmd

B, S, D, H = 2, 2048, 1024, 16
OL = 256          # local projection dims (4 heads x 64)
NI = D // 128     # contraction chunks for projections
NK = S // 128     # key chunks
NQ = S // 512     # query blocks

_CACHE = {}


def _build():
    DT = mybir.dt.float32r
    F32 = mybir.dt.float32
    AF = mybir.ActivationFunctionType

    nc = bacc.Bacc("TRN2", target_bir_lowering=False, debug=False, num_devices=8)

    qt_d = nc.dram_tensor("qt", [D, S], DT, kind="ExternalInput").ap() \
        .rearrange("(c p) s -> c p s", p=128)
    kt_d = nc.dram_tensor("kt", [D, S], DT, kind="ExternalInput").ap() \
        .rearrange("(c p) s -> c p s", p=128)
    vt_d = nc.dram_tensor("vt", [D, S], DT, kind="ExternalInput").ap() \
        .rearrange("(c p) s -> c p s", p=128)
    wq_d = nc.dram_tensor("wqt", [D, OL], DT, kind="ExternalInput").ap() \
        .rearrange("(c p) o -> c p o", p=128)
    wk_d = nc.dram_tensor("wkt", [D, OL], DT, kind="ExternalInput").ap() \
        .rearrange("(c p) o -> c p o", p=128)
    wv_d = nc.dram_tensor("wvt", [D, OL], DT, kind="ExternalInput").ap() \
        .rearrange("(c p) o -> c p o", p=128)
    bq_d = nc.dram_tensor("bq2", [2, 128, 1], F32, kind="ExternalInput").ap()
    bk_d = nc.dram_tensor("bk2", [2, 128, 1], F32, kind="ExternalInput").ap()
    bv_d = nc.dram_tensor("bv1", [1, OL], DT, kind="ExternalInput").ap()
    wo_d = nc.dram_tensor("wot", [OL, D], DT, kind="ExternalInput").ap() \
        .rearrange("(c p) o -> c p o", p=128)
    out_d = nc.dram_tensor("out_t", [D, S], F32, kind="ExternalOutput").ap() \
        .rearrange("(c p) s -> c p s", p=128)

    with tile.TileContext(nc) as tc:
        with (
            tc.tile_pool(name="per", bufs=1) as per,
            tc.tile_pool(name="wp", bufs=1) as wp,
            tc.tile_pool(name="ip", bufs=1) as ip,
            tc.tile_pool(name="pr", bufs=3) as pr,
            tc.tile_pool(name="sm", bufs=3) as sm,
            tc.tile_pool(name="ot", bufs=2) as ot,
            tc.tile_pool(name="osg", bufs=3) as osg,
            tc.tile_pool(name="pj", bufs=1, space="PSUM") as pj,
            tc.tile_pool(name="p1", bufs=2, space="PSUM") as p1,
            tc.tile_pool(name="px", bufs=2, space="PSUM") as px,
        ):
            # --- persistent tiles
            qt_sb = [per.tile([128, S], DT, tag=f"qt{m}", name=f"qt{m}")
                     for m in range(2)]
            kt_sb = [per.tile([128, S], DT, tag=f"kt{m}", name=f"kt{m}")
                     for m in range(2)]
            v_sb = [per.tile([128, 4, 65], DT, tag=f"v{sc}", name=f"v{sc}")
                    for sc in range(NK)]
            wo_sb = [per.tile([128, D], DT, tag=f"wo{c}", name=f"wo{c}")
                     for c in range(2)]
            bq_sb = [per.tile([128, 1], F32, tag=f"bq{m}", name=f"bq{m}")
                     for m in range(2)]
            bk_sb = [per.tile([128, 1], F32, tag=f"bk{m}", name=f"bk{m}")
                     for m in range(2)]
            bv_sb = per.tile([1, OL], DT, tag="bv", name="bv")
            ones_f = per.tile([1, 128], F32, tag="ones_f", name="ones_f")
            vones_f = per.tile([128, 1], F32, tag="vones_f", name="vones_f")
            ones_r = per.tile([1, 128], DT, tag="ones_r", name="ones_r")
            nc.vector.memset(ones_f[:], 1.0)
            nc.vector.memset(vones_f[:], 1.0)
            nc.vector.tensor_copy(ones_r[:], ones_f[:])

            for m in range(2):
                nc.sync.dma_start(bq_sb[m][:], bq_d[m])
                nc.sync.dma_start(bk_sb[m][:], bk_d[m])
            nc.sync.dma_start(bv_sb[:], bv_d)
            for c in range(2):
                nc.sync.dma_start(wo_sb[c][:], wo_d[c])

            def load_wa(w_dr, a_dr):
                ws, as_ = [], []
                for i in range(NI):
                    w = wp.tile([128, OL], DT, tag=f"w{i}", name=f"w{i}")
                    nc.sync.dma_start(w[:], w_dr[i])
                    ws.append(w)
                    a = ip.tile([128, S], DT, tag=f"a{i}", name=f"a{i}")
                    nc.sync.dma_start(a[:], a_dr[i])
                    as_.append(a)
                return ws, as_

            def proj_qk(w_dr, a_dr, bias_sb, dst_sb):
                # dst[o, s] = sum_i W[o, i] X[s, i]; i-streamed, 2 psum accs
                ws, as_ = load_wa(w_dr, a_dr)
                for m in range(2):
                    for sg in range(2):
                        accs = [pj.tile([128, 512], F32, tag=f"pj{j}",
                                        name=f"pj{j}") for j in range(2)]
                        for i in range(NI):
                            for j in range(2):
                                s = sg * 2 + j
                                nc.tensor.matmul(
                                    accs[j][:],
                                    ws[i][:, m * 128:(m + 1) * 128],
                                    as_[i][:, s * 512:(s + 1) * 512],
                                    start=(i == 0),
                                    stop=(i == NI - 1),
                                )
                        for j in range(2):
                            s = sg * 2 + j
                            nc.vector.tensor_scalar_add(
                                dst_sb[m][:, s * 512:(s + 1) * 512],
                                accs[j][:], bias_sb[m][:],
                            )

            def proj_v(w_dr, a_dr):
                # v[s, o] = sum_i X[s, i] W[o, i] + bv (bias via K=1 matmul)
                ws, as_ = load_wa(w_dr, a_dr)
                for grp in range(8):
                    accs = [pj.tile([128, OL], F32, tag=f"pj{j}",
                                    name=f"pj{j}") for j in range(2)]
                    for i in range(NI):
                        for j in range(2):
                            sc = grp * 2 + j
                            nc.tensor.matmul(
                                accs[j][:],
                                as_[i][:, sc * 128:(sc + 1) * 128],
                                ws[i][:],
                                start=(i == 0),
                                stop=False,
                            )
                    for j in range(2):
                        sc = grp * 2 + j
                        nc.tensor.matmul(
                            accs[j][:], ones_r[:], bv_sb[:],
                            start=False, stop=True,
                        )
                        for h in range(4):
                            nc.vector.tensor_copy(
                                v_sb[sc][:, h, 0:64],
                                accs[j][:, h * 64:(h + 1) * 64],
                            )
                        nc.vector.tensor_copy(
                            v_sb[sc][:, :, 64:65],
                            vones_f[:].to_broadcast((128, 4, 1)),
                        )

            proj_qk(wk_d, kt_d, bk_sb, kt_sb)
            proj_v(wv_d, vt_d)
            proj_qk(wq_d, qt_d, bq_sb, qt_sb)

            # --- attention + output projection, per query block
            for qb in range(NQ):
                qsl = slice(qb * 512, (qb + 1) * 512)
                ots = [ot.tile([128, 512], DT, tag=f"c{c}", name=f"otc{c}")
                       for c in range(2)]
                for pair in range(2):
                    acc = [px.tile([65, 512], F32, tag="x", name="acc")
                           for _ in range(2)]
                    pending = None  # software-pipelined PV matmuls
                    for kc in range(NK):
                        ksl = slice(kc * 128, (kc + 1) * 128)
                        ps1 = p1.tile([128, 1024], F32, tag="s", name="s")
                        for hh in range(2):
                            psl = slice(hh * 64, (hh + 1) * 64)
                            nc.tensor.matmul(
                                ps1[:, hh * 512:(hh + 1) * 512],
                                kt_sb[pair][psl, ksl],
                                qt_sb[pair][psl, qsl],
                                start=True, stop=True,
                            )
                        prob = pr.tile([128, 1024], DT, tag="p", name="p")
                        nc.scalar.activation(
                            prob[:], ps1[:], AF.Exp, scale=0.125
                        )
                        if pending is not None:
                            pkc, pprob = pending
                            for hh in range(2):
                                nc.tensor.matmul(
                                    acc[hh][:], v_sb[pkc][:, pair * 2 + hh, :],
                                    pprob[:, hh * 512:(hh + 1) * 512],
                                    start=(pkc == 0), stop=(pkc == NK - 1),
                                )
                        pending = (kc, prob)
                    pkc, pprob = pending
                    for hh in range(2):
                        nc.tensor.matmul(
                            acc[hh][:], v_sb[pkc][:, pair * 2 + hh, :],
                            pprob[:, hh * 512:(hh + 1) * 512],
                            start=(pkc == 0), stop=(pkc == NK - 1),
                        )
                    # normalize off-bank: copy out fast, then 1/denom bcast
                    for hh in range(2):
                        un = sm.tile([64, 512], F32, tag="un", name="un")
                        nc.vector.tensor_copy(un[:], acc[hh][0:64, :])
                        den = sm.tile([1, 512], F32, tag="den", name="den")
                        nc.vector.tensor_copy(den[:], acc[hh][64:65, :])
                        rec = sm.tile([1, 512], F32, tag="rec", name="rec")
                        nc.vector.reciprocal_approx_fast(rec[:], den[:])
                        rb = sm.tile([64, 512], F32, tag="rb", name="rb")
                        nc.gpsimd.partition_broadcast(rb[:], rec[:])
                        nc.vector.tensor_mul(
                            ots[pair][hh * 64:(hh + 1) * 64, :],
                            un[:], rb[:],
                        )
                # output projection for this query block
                for oc in range(8):
                    osl = slice(oc * 128, (oc + 1) * 128)
                    pso = px.tile([128, 512], F32, tag="x", name="pso")
                    for c in range(2):
                        nc.tensor.matmul(
                            pso[:], wo_sb[c][:, osl], ots[c][:],
                            start=(c == 0), stop=(c == 1),
                        )
                    st = osg.tile([128, 512], F32, tag="st", name="st")
                    nc.vector.tensor_copy(st[:], pso[:])
                    nc.sync.dma_start(out_d[oc][:, qsl], st[:])

    nc.compile()
    return nc


def _get_nc():
    if "nc" not in _CACHE:
        _CACHE["nc"] = _build()
    return _CACHE["nc"]


def kernel(Q, K, V, Wq, bq, Wk, bk, Wv, bv, Wo, bo):
    nc = _get_nc()
    f = np.float32
    in_maps = []
    for core in range(8):
        b, g = divmod(core, 4)
        sl = slice(g * OL, (g + 1) * OL)
        in_maps.append({
            "qt": np.ascontiguousarray(Q[b].T, dtype=f),
            "kt": np.ascontiguousarray(K[b].T, dtype=f),
            "vt": np.ascontiguousarray(V[b].T, dtype=f),
            "wqt": np.ascontiguousarray(Wq[sl].T, dtype=f),
            "wkt": np.ascontiguousarray(Wk[sl].T, dtype=f),
            "wvt": np.ascontiguousarray(Wv[sl].T, dtype=f),
            "bq2": np.ascontiguousarray(bq[sl].reshape(2, 128, 1), dtype=f),
            "bk2": np.ascontiguousarray(bk[sl].reshape(2, 128, 1), dtype=f),
            "bv1": np.ascontiguousarray(bv[sl].reshape(1, OL), dtype=f),
            "wot": np.ascontiguousarray(Wo[:, sl].T, dtype=f),
        })
    res = run_bass_kernel_spmd(nc, in_maps, core_ids=list(range(8)))
    out = np.empty((B, S, D), np.float32)
    for b in range(B):
        acc = res.results[b * 4 + 0]["out_t"].astype(np.float64)
        for g in range(1, 4):
            acc += res.results[b * 4 + g]["out_t"]
        out[b] = (acc.T + bo).astype(np.float32)
    return out


# revision 6
# speedup vs baseline: 1.5701x; 1.0975x over previous
"""Multi-head attention (B=2, S=2048, D=1024, H=16) on 8 Trainium2 NeuronCores.

Sharding: core c -> batch b = c // 4, head-group g = c % 4 (4 heads = 256 proj
dims per core). Each core computes its 4 heads' attention plus the matching
slice of the output projection; the host sums the 4 partial outputs per batch
and adds bo.

Device layouts (matmul operands float32r = fp32 bits at bf16 PE rate):
  qT/kT [o, s]   : proj from host-transposed Q/K (contraction on partitions)
  v     [s, o]   : natural layout + ones column per head (softmax denominator
                   rides along row 64 of the PV matmul output)
  scoresT [k, q] : head pairs row-packed on the PE (base_partition 0/64);
                   both halves of a [128,1024] PSUM tile -> one wide Exp
  outT  [d, q]   : unnormalized; moved off PSUM fast, normalized with
                   reciprocal_approx_fast + GpSimd partition_broadcast
  out_pT [o, q]  : local slice of x @ Wo.T; host transposes + sums + bias
"""

import numpy as np

import concourse.bass as bass
import concourse.mybir as mybir
import concourse.tile as tile
from concourse import bacc
from concourse.bass_utils import run_bass_kernel_spmd

B, S, D, H = 2, 2048, 1024, 16
OL = 256          # local projection dims (4 heads x 64)
NI = D // 128     # contraction chunks for projections
NK = S // 128     # key chunks
NQ = S // 512     # query blocks

_CACHE = {}


def _build():
    DT = mybir.dt.float32r
    F32 = mybir.dt.float32
    AF = mybir.ActivationFunctionType

    nc = bacc.Bacc("TRN2", target_bir_lowering=False, debug=False, num_devices=8)

    qt_d = nc.dram_tensor("qt", [D, S], DT, kind="ExternalInput").ap() \
        .rearrange("(c p) s -> c p s", p=128)
    kt_d = nc.dram_tensor("kt", [D, S], DT, kind="ExternalInput").ap() \
        .rearrange("(c p) s -> c p s", p=128)
    vt_d = nc.dram_tensor("vt", [D, S], DT, kind="ExternalInput").ap() \
        .rearrange("(c p) s -> c p s", p=128)
    wq_d = nc.dram_tensor("wqt", [D, OL], DT, kind="ExternalInput").ap() \
        .rearrange("(c p) o -> c p o", p=128)
    wk_d = nc.dram_tensor("wkt", [D, OL], DT, kind="ExternalInput").ap() \
        .rearrange("(c p) o -> c p o", p=128)
    wv_d = nc.dram_tensor("wvt", [D, OL], DT, kind="ExternalInput").ap() \
        .rearrange("(c p) o -> c p o", p=128)
    bq_d = nc.dram_tensor("bq2", [2, 128, 1], F32, kind="ExternalInput").ap()
    bk_d = nc.dram_tensor("bk2", [2, 128, 1], F32, kind="ExternalInput").ap()
    bv_d = nc.dram_tensor("bv1", [1, OL], DT, kind="ExternalInput").ap()
    wo_d = nc.dram_tensor("wot", [OL, D], DT, kind="ExternalInput").ap() \
        .rearrange("(c p) o -> c p o", p=128)
    out_d = nc.dram_tensor("out_t", [D, S], F32, kind="ExternalOutput").ap() \
        .rearrange("(c p) s -> c p s", p=128)

    with tile.TileContext(nc) as tc:
        with (
            tc.tile_pool(name="per", bufs=1) as per,
            tc.tile_pool(name="wp", bufs=1) as wp,
            tc.tile_pool(name="ip", bufs=1) as ip,
            tc.tile_pool(name="pr", bufs=3) as pr,
            tc.tile_pool(name="sm", bufs=3) as sm,
            tc.tile_pool(name="ot", bufs=2) as ot,
            tc.tile_pool(name="osg", bufs=3) as osg,
            tc.tile_pool(name="pj", bufs=2, space="PSUM") as pj,
            tc.tile_pool(name="p1", bufs=2, space="PSUM") as p1,
            tc.tile_pool(name="px", bufs=2, space="PSUM") as px,
        ):
            # --- persistent tiles
            qt_sb = [per.tile([128, S], DT, tag=f"qt{m}", name=f"qt{m}")
                     for m in range(2)]
            kt_sb = [per.tile([128, S], DT, tag=f"kt{m}", name=f"kt{m}")
                     for m in range(2)]
            v_sb = [per.tile([128, 4, 65], DT, tag=f"v{sc}", name=f"v{sc}")
                    for sc in range(NK)]
            wo_sb = [per.tile([128, D], DT, tag=f"wo{c}", name=f"wo{c}")
                     for c in range(2)]
            bq_sb = [per.tile([128, 1], F32, tag=f"bq{m}", name=f"bq{m}")
                     for m in range(2)]
            bk_sb = [per.tile([128, 1], F32, tag=f"bk{m}", name=f"bk{m}")
                     for m in range(2)]
            bv_sb = per.tile([1, OL], DT, tag="bv", name="bv")
            ones_f = per.tile([1, 128], F32, tag="ones_f", name="ones_f")
            vones_f = per.tile([128, 1], F32, tag="vones_f", name="vones_f")
            ones_r = per.tile([1, 128], DT, tag="ones_r", name="ones_r")
            nc.vector.memset(ones_f[:], 1.0)
            nc.vector.memset(vones_f[:], 1.0)
            nc.vector.tensor_copy(ones_r[:], ones_f[:])

            for m in range(2):
                nc.sync.dma_start(bq_sb[m][:], bq_d[m])
                nc.sync.dma_start(bk_sb[m][:], bk_d[m])
            nc.sync.dma_start(bv_sb[:], bv_d)
            for c in range(2):
                nc.sync.dma_start(wo_sb[c][:], wo_d[c])

            def load_wa(w_dr, a_dr):
                ws, as_ = [], []
                for i in range(NI):
                    w = wp.tile([128, OL], DT, tag=f"w{i}", name=f"w{i}")
                    nc.sync.dma_start(w[:], w_dr[i])
                    ws.append(w)
                    a = ip.tile([128, S], DT, tag=f"a{i}", name=f"a{i}")
                    nc.sync.dma_start(a[:], a_dr[i])
                    as_.append(a)
                return ws, as_

            def proj_qk(w_dr, a_dr, bias_sb, dst_sb):
                # dst[o, s] = sum_i W[o, i] X[s, i]; i-streamed, 2 psum accs
                ws, as_ = load_wa(w_dr, a_dr)
                for m in range(2):
                    for s in range(4):
                        acc = pj.tile([128, 512], F32, tag="pj", name="pj")
                        for i in range(NI):
                            nc.tensor.matmul(
                                acc[:],
                                ws[i][:, m * 128:(m + 1) * 128],
                                as_[i][:, s * 512:(s + 1) * 512],
                                start=(i == 0),
                                stop=(i == NI - 1),
                            )
                        nc.vector.tensor_scalar_add(
                            dst_sb[m][:, s * 512:(s + 1) * 512],
                            acc[:], bias_sb[m][:],
                        )

            def proj_v(w_dr, a_dr):
                # v[s, o] = sum_i X[s, i] W[o, i] + bv (bias via K=1 matmul)
                ws, as_ = load_wa(w_dr, a_dr)
                for sc in range(NK):
                    acc = pj.tile([128, OL], F32, tag="pj", name="pj")
                    for i in range(NI):
                        nc.tensor.matmul(
                            acc[:],
                            as_[i][:, sc * 128:(sc + 1) * 128],
                            ws[i][:],
                            start=(i == 0),
                            stop=False,
                        )
                    nc.tensor.matmul(
                        acc[:], ones_r[:], bv_sb[:],
                        start=False, stop=True,
                    )
                    for h in range(4):
                        nc.vector.tensor_copy(
                            v_sb[sc][:, h, 0:64],
                            acc[:, h * 64:(h + 1) * 64],
                        )
                    nc.vector.tensor_copy(
                        v_sb[sc][:, :, 64:65],
                        vones_f[:].to_broadcast((128, 4, 1)),
                    )

            proj_qk(wk_d, kt_d, bk_sb, kt_sb)
            proj_v(wv_d, vt_d)
            proj_qk(wq_d, qt_d, bq_sb, qt_sb)

            # --- attention + output projection, per query block
            # OP of qb-1 is spread into qb's pair-0 kc loop (PE slack there);
            # PV matmuls trail the exp by 2 kc steps so the PE never waits.
            def emit_op(qb, ots_prev):
                ops = []
                for oc in range(8):
                    osl = slice(oc * 128, (oc + 1) * 128)
                    pso = p1.tile([128, 512], F32, tag="s", name="pso")
                    for c in range(2):
                        nc.tensor.matmul(
                            pso[:], wo_sb[c][:, osl], ots_prev[c][:],
                            start=(c == 0), stop=(c == 1),
                        )
                    st = osg.tile([128, 512], F32, tag="st", name="st")
                    nc.vector.tensor_copy(st[:], pso[:])
                    nc.sync.dma_start(
                        out_d[oc][:, qb * 512:(qb + 1) * 512], st[:])
                    ops.append(None)

            ots_prev = None
            for qb in range(NQ):
                qsl = slice(qb * 512, (qb + 1) * 512)
                ots = [ot.tile([128, 512], DT, tag=f"c{c}", name=f"otc{c}")
                       for c in range(2)]
                for pair in range(2):
                    acc = [px.tile([65, 512], F32, tag="x", name="acc")
                           for _ in range(2)]
                    pend = []
                    op_iter = None
                    if pair == 0 and ots_prev is not None:
                        op_iter = iter(range(8))
                    for kc in range(NK):
                        ksl = slice(kc * 128, (kc + 1) * 128)
                        ps1 = p1.tile([128, 1024], F32, tag="s", name="s")
                        for hh in range(2):
                            psl = slice(hh * 64, (hh + 1) * 64)
                            nc.tensor.matmul(
                                ps1[:, hh * 512:(hh + 1) * 512],
                                kt_sb[pair][psl, ksl],
                                qt_sb[pair][psl, qsl],
                                start=True, stop=True,
                            )
                        prob = pr.tile([128, 1024], DT, tag="p", name="p")
                        nc.scalar.activation(
                            prob[:], ps1[:], AF.Exp, scale=0.125
                        )
                        pend.append((kc, prob))
                        if len(pend) > 2:
                            pkc, pprob = pend.pop(0)
                            for hh in range(2):
                                nc.tensor.matmul(
                                    acc[hh][:], v_sb[pkc][:, pair * 2 + hh, :],
                                    pprob[:, hh * 512:(hh + 1) * 512],
                                    start=(pkc == 0), stop=(pkc == NK - 1),
                                )
                        if op_iter is not None and kc % 2 == 1:
                            oc = next(op_iter, None)
                            if oc is not None:
                                osl = slice(oc * 128, (oc + 1) * 128)
                                pso = p1.tile([128, 512], F32, tag="s",
                                              name="pso")
                                for c in range(2):
                                    nc.tensor.matmul(
                                        pso[:], wo_sb[c][:, osl],
                                        ots_prev[c][:],
                                        start=(c == 0), stop=(c == 1),
                                    )
                                st = osg.tile([128, 512], F32, tag="st",
                                              name="st")
                                nc.vector.tensor_copy(st[:], pso[:])
                                nc.sync.dma_start(
                                    out_d[oc][:, (qb - 1) * 512:qb * 512],
                                    st[:])
                    for pkc, pprob in pend:
                        for hh in range(2):
                            nc.tensor.matmul(
                                acc[hh][:], v_sb[pkc][:, pair * 2 + hh, :],
                                pprob[:, hh * 512:(hh + 1) * 512],
                                start=(pkc == 0), stop=(pkc == NK - 1),
                            )
                    # normalize off-bank: copy out fast, then 1/denom bcast
                    for hh in range(2):
                        un = sm.tile([64, 512], F32, tag="un", name="un")
                        nc.vector.tensor_copy(un[:], acc[hh][0:64, :])
                        den = sm.tile([1, 512], F32, tag="den", name="den")
                        nc.vector.tensor_copy(den[:], acc[hh][64:65, :])
                        rec = sm.tile([1, 512], F32, tag="rec", name="rec")
                        nc.vector.reciprocal_approx_fast(rec[:], den[:])
                        rb = sm.tile([64, 512], F32, tag="rb", name="rb")
                        nc.gpsimd.partition_broadcast(rb[:], rec[:])
                        nc.vector.tensor_mul(
                            ots[pair][hh * 64:(hh + 1) * 64, :],
                            un[:], rb[:],
                        )
                ots_prev = ots
            emit_op(NQ - 1, ots_prev)

    nc.compile()
    return nc


def _get_nc():
    if "nc" not in _CACHE:
        _CACHE["nc"] = _build()
    return _CACHE["nc"]


def kernel(Q, K, V, Wq, bq, Wk, bk, Wv, bv, Wo, bo):
    nc = _get_nc()
    f = np.float32
    in_maps = []
    for core in range(8):
        b, g = divmod(core, 4)
        sl = slice(g * OL, (g + 1) * OL)
        in_maps.append({
            "qt": np.ascontiguousarray(Q[b].T, dtype=f),
            "kt": np.ascontiguousarray(K[b].T, dtype=f),
            "vt": np.ascontiguousarray(V[b].T, dtype=f),
            "wqt": np.ascontiguousarray(Wq[sl].T, dtype=f),
            "wkt": np.ascontiguousarray(Wk[sl].T, dtype=f),
            "wvt": np.ascontiguousarray(Wv[sl].T, dtype=f),
            "bq2": np.ascontiguousarray(bq[sl].reshape(2, 128, 1), dtype=f),
            "bk2": np.ascontiguousarray(bk[sl].reshape(2, 128, 1), dtype=f),
            "bv1": np.ascontiguousarray(bv[sl].reshape(1, OL), dtype=f),
            "wot": np.ascontiguousarray(Wo[:, sl].T, dtype=f),
        })
    res = run_bass_kernel_spmd(nc, in_maps, core_ids=list(range(8)))
    out = np.empty((B, S, D), np.float32)
    for b in range(B):
        acc = res.results[b * 4 + 0]["out_t"].astype(np.float64)
        for g in range(1, 4):
            acc += res.results[b * 4 + g]["out_t"]
        out[b] = (acc.T + bo).astype(np.float32)
    return out


# revision 7
# speedup vs baseline: 1.9066x; 1.2143x over previous
"""Multi-head attention (B=2, S=2048, D=1024, H=16) on 8 Trainium2 NeuronCores.

Sharding: core c -> batch b = c // 4, head-group g = c % 4 (4 heads = 256 proj
dims per core). Each core computes its 4 heads' attention plus the matching
slice of the output projection; the host sums the 4 partial outputs per batch
and adds bo.

Device layouts (matmul operands float32r = fp32 bits at bf16 PE rate):
  qT/kT [o, s]   : proj from host-transposed Q/K (contraction on partitions)
  v     [s, o]   : natural layout + ones column per head (softmax denominator
                   rides along row 64 of the PV matmul output)
  scoresT [k, q] : head pairs row-packed on the PE (base_partition 0/64);
                   both halves of a [128,1024] PSUM tile -> one wide Exp
  outT  [d, q]   : unnormalized; moved off PSUM fast, normalized with
                   reciprocal_approx_fast + GpSimd partition_broadcast
  out_pT [o, q]  : local slice of x @ Wo.T; host transposes + sums + bias
"""

import ml_dtypes
import numpy as np

import concourse.bass as bass
import concourse.mybir as mybir
import concourse.tile as tile
from concourse import bacc
from concourse.bass_utils import run_bass_kernel_spmd

B, S, D, H = 2, 2048, 1024, 16
OL = 256          # local projection dims (4 heads x 64)
NI = D // 128     # contraction chunks for projections
NK = S // 128     # key chunks
NQ = S // 512     # query blocks

_CACHE = {}


def _build():
    DT = mybir.dt.bfloat16
    F32 = mybir.dt.float32
    AF = mybir.ActivationFunctionType

    nc = bacc.Bacc("TRN2", target_bir_lowering=False, debug=False, num_devices=8)

    qt_d = nc.dram_tensor("qt", [D, S], DT, kind="ExternalInput").ap() \
        .rearrange("(c p) s -> c p s", p=128)
    kt_d = nc.dram_tensor("kt", [D, S], DT, kind="ExternalInput").ap() \
        .rearrange("(c p) s -> c p s", p=128)
    vt_d = nc.dram_tensor("vt", [D, S], DT, kind="ExternalInput").ap() \
        .rearrange("(c p) s -> c p s", p=128)
    wq_d = nc.dram_tensor("wqt", [D, OL], DT, kind="ExternalInput").ap() \
        .rearrange("(c p) o -> c p o", p=128)
    wk_d = nc.dram_tensor("wkt", [D, OL], DT, kind="ExternalInput").ap() \
        .rearrange("(c p) o -> c p o", p=128)
    wv_d = nc.dram_tensor("wvt", [D, OL], DT, kind="ExternalInput").ap() \
        .rearrange("(c p) o -> c p o", p=128)
    bq_d = nc.dram_tensor("bq2", [2, 128, 1], F32, kind="ExternalInput").ap()
    bk_d = nc.dram_tensor("bk2", [2, 128, 1], F32, kind="ExternalInput").ap()
    bv_d = nc.dram_tensor("bv1", [1, OL], DT, kind="ExternalInput").ap()
    wo_d = nc.dram_tensor("wot", [OL, D], DT, kind="ExternalInput").ap() \
        .rearrange("(c p) o -> c p o", p=128)
    out_d = nc.dram_tensor("out_t", [D, S], F32, kind="ExternalOutput").ap() \
        .rearrange("(c p) s -> c p s", p=128)

    with tile.TileContext(nc) as tc:
        with (
            tc.tile_pool(name="per", bufs=1) as per,
            tc.tile_pool(name="wp", bufs=1) as wp,
            tc.tile_pool(name="ip", bufs=1) as ip,
            tc.tile_pool(name="pr", bufs=3) as pr,
            tc.tile_pool(name="sm", bufs=3) as sm,
            tc.tile_pool(name="ot", bufs=2) as ot,
            tc.tile_pool(name="osg", bufs=3) as osg,
            tc.tile_pool(name="pj", bufs=2, space="PSUM") as pj,
            tc.tile_pool(name="p1", bufs=2, space="PSUM") as p1,
            tc.tile_pool(name="px", bufs=2, space="PSUM") as px,
        ):
            # --- persistent tiles
            qt_sb = [per.tile([128, S], DT, tag=f"qt{m}", name=f"qt{m}")
                     for m in range(2)]
            kt_sb = [per.tile([128, S], DT, tag=f"kt{m}", name=f"kt{m}")
                     for m in range(2)]
            v_sb = [per.tile([128, 4, 65], DT, tag=f"v{sc}", name=f"v{sc}")
                    for sc in range(NK)]
            wo_sb = [per.tile([128, D], DT, tag=f"wo{c}", name=f"wo{c}")
                     for c in range(2)]
            bq_sb = [per.tile([128, 1], F32, tag=f"bq{m}", name=f"bq{m}")
                     for m in range(2)]
            bk_sb = [per.tile([128, 1], F32, tag=f"bk{m}", name=f"bk{m}")
                     for m in range(2)]
            bv_sb = per.tile([1, OL], DT, tag="bv", name="bv")
            ones_f = per.tile([1, 128], F32, tag="ones_f", name="ones_f")
            vones_f = per.tile([128, 1], F32, tag="vones_f", name="vones_f")
            ones_r = per.tile([1, 128], DT, tag="ones_r", name="ones_r")
            nc.vector.memset(ones_f[:], 1.0)
            nc.vector.memset(vones_f[:], 1.0)
            nc.vector.tensor_copy(ones_r[:], ones_f[:])

            for m in range(2):
                nc.sync.dma_start(bq_sb[m][:], bq_d[m])
                nc.sync.dma_start(bk_sb[m][:], bk_d[m])
            nc.sync.dma_start(bv_sb[:], bv_d)
            for c in range(2):
                nc.sync.dma_start(wo_sb[c][:], wo_d[c])

            def load_wa(w_dr, a_dr):
                ws, as_ = [], []
                for i in range(NI):
                    w = wp.tile([128, OL], DT, tag=f"w{i}", name=f"w{i}")
                    nc.sync.dma_start(w[:], w_dr[i])
                    ws.append(w)
                    a = ip.tile([128, S], DT, tag=f"a{i}", name=f"a{i}")
                    nc.sync.dma_start(a[:], a_dr[i])
                    as_.append(a)
                return ws, as_

            def proj_qk(w_dr, a_dr, bias_sb, dst_sb):
                # dst[o, s] = sum_i W[o, i] X[s, i]; i-streamed, 2 psum accs
                ws, as_ = load_wa(w_dr, a_dr)
                for m in range(2):
                    for s in range(4):
                        acc = pj.tile([128, 512], F32, tag="pj", name="pj")
                        for i in range(NI):
                            nc.tensor.matmul(
                                acc[:],
                                ws[i][:, m * 128:(m + 1) * 128],
                                as_[i][:, s * 512:(s + 1) * 512],
                                start=(i == 0),
                                stop=(i == NI - 1),
                            )
                        nc.vector.tensor_scalar_add(
                            dst_sb[m][:, s * 512:(s + 1) * 512],
                            acc[:], bias_sb[m][:],
                        )

            def proj_v(w_dr, a_dr):
                # v[s, o] = sum_i X[s, i] W[o, i] + bv (bias via K=1 matmul)
                ws, as_ = load_wa(w_dr, a_dr)
                for sc in range(NK):
                    acc = pj.tile([128, OL], F32, tag="pj", name="pj")
                    for i in range(NI):
                        nc.tensor.matmul(
                            acc[:],
                            as_[i][:, sc * 128:(sc + 1) * 128],
                            ws[i][:],
                            start=(i == 0),
                            stop=False,
                        )
                    nc.tensor.matmul(
                        acc[:], ones_r[:], bv_sb[:],
                        start=False, stop=True,
                    )
                    for h in range(4):
                        nc.vector.tensor_copy(
                            v_sb[sc][:, h, 0:64],
                            acc[:, h * 64:(h + 1) * 64],
                        )
                    nc.vector.tensor_copy(
                        v_sb[sc][:, :, 64:65],
                        vones_f[:].to_broadcast((128, 4, 1)),
                    )

            proj_qk(wk_d, kt_d, bk_sb, kt_sb)
            proj_v(wv_d, vt_d)
            proj_qk(wq_d, qt_d, bq_sb, qt_sb)

            # --- attention + output projection, per query block
            # OP of qb-1 is spread into qb's pair-0 kc loop (PE slack there);
            # PV matmuls trail the exp by 2 kc steps so the PE never waits.
            def emit_op(qb, ots_prev):
                ops = []
                for oc in range(8):
                    osl = slice(oc * 128, (oc + 1) * 128)
                    pso = p1.tile([128, 512], F32, tag="s", name="pso")
                    for c in range(2):
                        nc.tensor.matmul(
                            pso[:], wo_sb[c][:, osl], ots_prev[c][:],
                            start=(c == 0), stop=(c == 1),
                        )
                    st = osg.tile([128, 512], F32, tag="st", name="st")
                    nc.vector.tensor_copy(st[:], pso[:])
                    nc.sync.dma_start(
                        out_d[oc][:, qb * 512:(qb + 1) * 512], st[:])
                    ops.append(None)

            ots_prev = None
            for qb in range(NQ):
                qsl = slice(qb * 512, (qb + 1) * 512)
                ots = [ot.tile([128, 512], DT, tag=f"c{c}", name=f"otc{c}")
                       for c in range(2)]
                for pair in range(2):
                    acc = [px.tile([65, 512], F32, tag="x", name="acc")
                           for _ in range(2)]
                    pend = []
                    op_iter = None
                    if pair == 0 and ots_prev is not None:
                        op_iter = iter(range(8))
                    for kc in range(NK):
                        ksl = slice(kc * 128, (kc + 1) * 128)
                        ps1 = p1.tile([128, 1024], F32, tag="s", name="s")
                        for hh in range(2):
                            psl = slice(hh * 64, (hh + 1) * 64)
                            nc.tensor.matmul(
                                ps1[:, hh * 512:(hh + 1) * 512],
                                kt_sb[pair][psl, ksl],
                                qt_sb[pair][psl, qsl],
                                start=True, stop=True,
                            )
                        prob = pr.tile([128, 1024], DT, tag="p", name="p")
                        nc.scalar.activation(
                            prob[:], ps1[:], AF.Exp, scale=0.125
                        )
                        pend.append((kc, prob))
                        if len(pend) > 2:
                            pkc, pprob = pend.pop(0)
                            for hh in range(2):
                                nc.tensor.matmul(
                                    acc[hh][:], v_sb[pkc][:, pair * 2 + hh, :],
                                    pprob[:, hh * 512:(hh + 1) * 512],
                                    start=(pkc == 0), stop=(pkc == NK - 1),
                                )
                        if op_iter is not None and kc % 2 == 1:
                            oc = next(op_iter, None)
                            if oc is not None:
                                osl = slice(oc * 128, (oc + 1) * 128)
                                pso = p1.tile([128, 512], F32, tag="s",
                                              name="pso")
                                for c in range(2):
                                    nc.tensor.matmul(
                                        pso[:], wo_sb[c][:, osl],
                                        ots_prev[c][:],
                                        start=(c == 0), stop=(c == 1),
                                    )
                                st = osg.tile([128, 512], F32, tag="st",
                                              name="st")
                                nc.vector.tensor_copy(st[:], pso[:])
                                nc.sync.dma_start(
                                    out_d[oc][:, (qb - 1) * 512:qb * 512],
                                    st[:])
                    for pkc, pprob in pend:
                        for hh in range(2):
                            nc.tensor.matmul(
                                acc[hh][:], v_sb[pkc][:, pair * 2 + hh, :],
                                pprob[:, hh * 512:(hh + 1) * 512],
                                start=(pkc == 0), stop=(pkc == NK - 1),
                            )
                    # normalize off-bank: free both acc banks first
                    uns, dens = [], []
                    for hh in range(2):
                        un = sm.tile([64, 512], F32, tag=f"un{hh}",
                                     name=f"un{hh}")
                        nc.vector.tensor_copy(un[:], acc[hh][0:64, :])
                        den = sm.tile([1, 512], F32, tag=f"den{hh}",
                                      name=f"den{hh}")
                        nc.vector.tensor_copy(den[:], acc[hh][64:65, :])
                        uns.append(un)
                        dens.append(den)
                    for hh in range(2):
                        rec = sm.tile([1, 512], F32, tag="rec", name="rec")
                        nc.vector.reciprocal_approx_fast(rec[:], dens[hh][:])
                        rb = sm.tile([64, 512], F32, tag="rb", name="rb")
                        nc.gpsimd.partition_broadcast(rb[:], rec[:])
                        nc.vector.tensor_mul(
                            ots[pair][hh * 64:(hh + 1) * 64, :],
                            uns[hh][:], rb[:],
                        )
                ots_prev = ots
            emit_op(NQ - 1, ots_prev)

    nc.compile()
    return nc


def _get_nc():
    if "nc" not in _CACHE:
        _CACHE["nc"] = _build()
    return _CACHE["nc"]


def kernel(Q, K, V, Wq, bq, Wk, bk, Wv, bv, Wo, bo):
    nc = _get_nc()
    f = np.float32
    bf = ml_dtypes.bfloat16
    in_maps = []
    for core in range(8):
        b, g = divmod(core, 4)
        sl = slice(g * OL, (g + 1) * OL)
        in_maps.append({
            "qt": np.ascontiguousarray(Q[b].T, dtype=bf),
            "kt": np.ascontiguousarray(K[b].T, dtype=bf),
            "vt": np.ascontiguousarray(V[b].T, dtype=bf),
            "wqt": np.ascontiguousarray(Wq[sl].T, dtype=bf),
            "wkt": np.ascontiguousarray(Wk[sl].T, dtype=bf),
            "wvt": np.ascontiguousarray(Wv[sl].T, dtype=bf),
            "bq2": np.ascontiguousarray(bq[sl].reshape(2, 128, 1), dtype=f),
            "bk2": np.ascontiguousarray(bk[sl].reshape(2, 128, 1), dtype=f),
            "bv1": np.ascontiguousarray(bv[sl].reshape(1, OL), dtype=bf),
            "wot": np.ascontiguousarray(Wo[:, sl].T, dtype=bf),
        })
    res = run_bass_kernel_spmd(nc, in_maps, core_ids=list(range(8)))
    out = np.empty((B, S, D), np.float32)
    for b in range(B):
        acc = res.results[b * 4 + 0]["out_t"].astype(np.float64)
        for g in range(1, 4):
            acc += res.results[b * 4 + g]["out_t"]
        out[b] = (acc.T + bo).astype(np.float32)
    return out
